# revision 1
# baseline (speedup 1.0000x reference)
"""Trainium2 Bass kernel for ViTDet-style attention with decomposed
relative-position bias.

Problem shapes (hardcoded):
  x: (4, 32, 32, 768) f32, Wqkv: (768, 2304), Wproj: (768, 768),
  bproj: (768,), rel_pos_h/w: (63, 64).
  12 heads, head_dim 64, S = 32*32 = 1024.

Sharding: 48 (batch, head) pairs -> 6 heads per core, all of one batch per
core-pair. Each core computes its heads' attention and a partial output
projection (its heads' channel rows of Wproj); the host sums the two
partials per batch and adds bproj.

Device algorithm per core (bf16 matmuls, fp32 PSUM accumulation):
  - qkT = Wqk^T @ x^T  (x^T supplied pre-transposed by host; k pre-scaled)
  - v   = x @ Wv       (natural layout, with an appended ones column)
  - PhT = rel_pos_h^T @ qT; band-extract BhT[kh',(h,w)] = PhT[kh'+h,(h,w)]
    on the PE via shifted-identity selection matmuls (same for W axis)
  - scoresT (k x q) = kaugT^T @ qaugT in ONE K=128 matmul per tile:
    aug rows 0-63 = kT / qT, 64-95 = one-hot(h) / BhT, 96-127 = one-hot(w)/BwT
    => rel-pos bias folded into the QK matmul for free.
  - eT = exp(scoresT) on ScalarE (no max subtraction; scores are O(1)).
  - avT (65 x q) accumulates v_aug^T-matmul over k blocks; row 64 = softmax
    denominator via the ones column.
  - normalize via DVE reciprocal + gpsimd partition-broadcast + DVE multiply.
  - partial = out_heads @ Wproj_shard  (natural layout, DMA PSUM->DRAM).
"""

import numpy as np

import concourse.bass as bass
import concourse.bacc as bacc
import concourse.mybir as mybir
import concourse.tile as tile
from concourse.bass_utils import run_bass_kernel_spmd

F32 = mybir.dt.float32
F32R = mybir.dt.float32r
BF16 = mybir.dt.bfloat16

NH = 12          # total heads
C = 768
HD = 64
H = W = 32
S = H * W        # 1024
B = 4
NCORES = 8
HPC = NH * B // NCORES   # heads per core = 6
NCH = 6                  # C // 128 input-channel chunks
NKB = S // 128           # 8 k blocks
NQB = S // 128           # 8 q blocks
NHALF = 512              # matmul moving-dim half


def _r(ap):
    # operands are already float32r-typed
    return ap


def build_program():
    nc = bacc.Bacc("TRN2", target_bir_lowering=False, debug=False)

    xT = nc.declare_dram_parameter("xT", [C, S], BF16, isOutput=False)
    wqk = nc.declare_dram_parameter("wqk", [C, 2 * HPC * HD], BF16, isOutput=False)
    wv = nc.declare_dram_parameter("wv", [C, HPC * HD], BF16, isOutput=False)
    wproj = nc.declare_dram_parameter("wproj", [HPC * HD, C], BF16, isOutput=False)
    rhT = nc.declare_dram_parameter("rhT", [HD, 2 * H - 1], BF16, isOutput=False)
    rwT = nc.declare_dram_parameter("rwT", [HD, 2 * W - 1], BF16, isOutput=False)
    onehot = nc.declare_dram_parameter("onehot", [65, S], BF16, isOutput=False)
    idband = nc.declare_dram_parameter("idband", [2 * H - 1, 3 * W - 1], BF16,
                                       isOutput=False)
    out = nc.declare_dram_parameter("out", [S, C], F32, isOutput=True)

    # small DRAM bounce buffers for the rowsum transpose (I/O tensors --
    # internal DRAM scratch is paged and much slower for strided DMAs)
    rs_dram = nc.declare_dram_parameter("rs_dram", [S], F32, isOutput=True)
    rc_dram = nc.declare_dram_parameter("rc_dram", [S], F32, isOutput=True)

    with tile.TileContext(nc) as tc:
        with (
            tc.tile_pool(name="persist", bufs=1) as persist,
            tc.tile_pool(name="psum_big", bufs=2, space="PSUM") as psum_big,
            tc.tile_pool(name="psum_av", bufs=2, space="PSUM") as psum_av,
            tc.tile_pool(name="et", bufs=3) as et_pool,
            tc.tile_pool(name="small", bufs=2) as small,
        ):
            # ---- persistent SBUF loads ----
            xT_sb = []
            for ci in range(NCH):
                t = persist.tile([128, S], BF16, tag=f"xT{ci}", name=f"xT{ci}")
                nc.sync.dma_start(t[:], xT[128 * ci:128 * (ci + 1), :])
                xT_sb.append(t)
            wqk_sb = []
            for ci in range(NCH):
                t = persist.tile([128, 2 * HPC * HD], BF16, tag=f"wqk{ci}", name=f"wqk{ci}")
                nc.sync.dma_start(t[:], wqk[128 * ci:128 * (ci + 1), :])
                wqk_sb.append(t)
            wv_sb = []
            for ci in range(NCH):
                t = persist.tile([128, HPC * HD], BF16, tag=f"wv{ci}", name=f"wv{ci}")
                nc.sync.dma_start(t[:], wv[128 * ci:128 * (ci + 1), :])
                wv_sb.append(t)
            wproj_sb = []
            for ci in range(HPC * HD // 128):
                t = persist.tile([128, C], BF16, tag=f"wproj{ci}", name=f"wproj{ci}")
                nc.sync.dma_start(t[:], wproj[128 * ci:128 * (ci + 1), :])
                wproj_sb.append(t)
            idb_sb = persist.tile([2 * H - 1, 3 * W - 1], BF16, tag="idb",
                                  name="idb_sb")
            nc.sync.dma_start(idb_sb[:], idband[:, :])
            rhT_sb = persist.tile([HD, 2 * H - 1], BF16, tag="rhT", name="rhT_sb")
            nc.sync.dma_start(rhT_sb[:], rhT[:, :])
            rwT_sb = persist.tile([HD, 2 * W - 1], BF16, tag="rwT", name="rwT_sb")
            nc.sync.dma_start(rwT_sb[:], rwT[:, :])

            # ---- one-hot template (65, S), host-supplied constant ----
            # rows 0-63: one-hot(h)/one-hot(w) reversed; row 64: all ones
            oh = persist.tile([65, S], BF16, tag="onehot", name="onehot")
            nc.sync.dma_start(oh[:], onehot[:, :])

            # ---- augmented k/q tiles (128, S) per head ----
            kaug = [persist.tile([128, S], BF16, tag=f"kaug{i}", name=f"kaug{i}") for i in range(HPC)]
            qaug = [persist.tile([128, S], BF16, tag=f"qaug{i}", name=f"qaug{i}") for i in range(HPC)]
            for i in range(HPC):
                nc.vector.tensor_copy(kaug[i][64:128, :], oh[0:64, :])

            # ---- v projection (natural) + ones column ----
            # v_sb[sb]: (128, 6*65) cols [65i..65i+64) = head i v, col 65i+64 = 1
            v_sb = [persist.tile([128, HPC * (HD + 1)], BF16, tag=f"v{sb}", name=f"v{sb}")
                    for sb in range(NKB)]
            for sb in range(NKB):
                vp = psum_big.tile([128, HPC * HD + HPC], F32, tag="big", name="vp")
                for ci in range(NCH):
                    nc.tensor.matmul(
                        vp[:, 0:HPC * HD],
                        _r(xT_sb[ci][:, 128 * sb:128 * (sb + 1)]),
                        _r(wv_sb[ci][:]),
                        start=(ci == 0), stop=(ci == NCH - 1))
                # ones columns via outer product of the ones row
                nc.tensor.matmul(vp[:, HPC * HD:HPC * HD + HPC],
                                 oh[64:65, 128 * sb:128 * (sb + 1)],
                                 oh[64:65, 0:HPC], start=True, stop=True)
                src = bass.AP(vp.tensor, vp[:].offset,
                              [vp[:].ap[0], [HD, HPC], [1, HD]])
                dst = bass.AP(v_sb[sb].tensor, v_sb[sb][:].offset,
                              [v_sb[sb][:].ap[0], [HD + 1, HPC], [1, HD]])
                nc.vector.tensor_copy(dst, src)
                ones_src = bass.AP(vp.tensor, vp[:].offset + HPC * HD,
                                   [vp[:].ap[0], [1, HPC]])
                ones_dst = bass.AP(v_sb[sb].tensor, v_sb[sb][:].offset + HD,
                                   [v_sb[sb][:].ap[0], [HD + 1, HPC]])
                nc.vector.tensor_copy(ones_dst, ones_src)

            # ---- qk projection (transposed layout) ----
            # qkT octile t covers oc rows [128t, 128t+128): t<3 -> q, t>=3 -> k
            for t in range(2 * HPC * HD // 128):
                qp = psum_big.tile([128, S], F32, tag="big", name="qp")
                for ci in range(NCH):
                    for nh in range(S // NHALF):
                        nc.tensor.matmul(
                            qp[:, NHALF * nh:NHALF * (nh + 1)],
                            _r(wqk_sb[ci][:, 128 * t:128 * (t + 1)]),
                            _r(xT_sb[ci][:, NHALF * nh:NHALF * (nh + 1)]),
                            start=(ci == 0), stop=(ci == NCH - 1))
                for sub in range(2):
                    head = (t % 3) * 2 + sub
                    dst = (qaug if t < 3 else kaug)[head]
                    if t < 3:
                        nc.scalar.copy(dst[0:64, :], qp[64 * sub:64 * sub + 64, :])
                    else:
                        nc.vector.tensor_copy(dst[0:64, :],
                                              qp[64 * sub:64 * sub + 64, :])

            # ---- per head: rel-pos tables -> band-gather into qaug ----
            for i in range(HPC):
                php = psum_big.tile([2 * H - 1, S], F32, tag="big", name="php")
                pwp = psum_big.tile([2 * W - 1, S], F32, tag="big", name="pwp")
                for nh in range(S // NHALF):
                    sl = slice(NHALF * nh, NHALF * (nh + 1))
                    nc.tensor.matmul(php[:, sl], _r(rhT_sb[:]),
                                     _r(qaug[i][0:64, sl]), start=True, stop=True)
                    nc.tensor.matmul(pwp[:, sl], _r(rwT_sb[:]),
                                     _r(qaug[i][0:64, sl]), start=True, stop=True)
                ph_sb = small.tile([2 * H - 1, S], BF16, tag="ph_sb",
                                   name="ph_sb", bufs=2)
                pw_sb = small.tile([2 * W - 1, S], BF16, tag="pw_sb",
                                   name="pw_sb", bufs=2)
                nc.scalar.copy(ph_sb[:], php[:])
                nc.vector.tensor_copy(pw_sb[:], pwp[:])
                # band-extract on PE: BhT_rev[kh', (h,w)] = PhT[kh'+h, (h,w)]
                # = sum_r idband[r, kh'+h] * PhT[r, (h,w)]  (idband = I_63)
                bhp = psum_big.tile([H, S], F32, tag="big", name="bhp")
                bwp = psum_big.tile([W, S], F32, tag="big", name="bwp")
                for h in range(H):
                    nc.tensor.matmul(bhp[:, W * h:W * (h + 1)],
                                     idb_sb[:, h:h + H],
                                     ph_sb[:, W * h:W * (h + 1)],
                                     start=True, stop=True)
                for w in range(W):
                    # w-major output block: bwp[kw', w*32+h] = PwT[kw'+w,(h,w)]
                    rhs_w = bass.AP(pw_sb.tensor, pw_sb[:].offset + w,
                                    [pw_sb[:].ap[0], [W, H]])
                    nc.tensor.matmul(bwp[:, H * w:H * (w + 1)],
                                     idb_sb[:, w:w + W], rhs_w,
                                     start=True, stop=True)
                nc.scalar.copy(qaug[i][64:96, :], bhp[:])
                # permute w-major back to (h, w) order during the copy
                bwp_perm = bass.AP(bwp.tensor, bwp[:].offset,
                                   [bwp[:].ap[0], [1, H], [H, W]])
                nc.vector.tensor_copy(qaug[i][96:128, :], bwp_perm)

            # ---- attention per head ----
            out_headsT = [persist.tile([128, S], BF16, tag=f"ohT{c}",
                                       name=f"ohT{c}")
                          for c in range(HPC * HD // 128)]
            for i in range(HPC):
                av = psum_av.tile([HD + 1, S], F32, tag="av", name="av")
                for kb in range(NKB):
                    sc = psum_big.tile([128, S], F32, tag="big", name="qp")
                    for nh in range(S // NHALF):
                        sl = slice(NHALF * nh, NHALF * (nh + 1))
                        nc.tensor.matmul(
                            sc[:, sl],
                            _r(kaug[i][:, 128 * kb:128 * (kb + 1)]),
                            _r(qaug[i][:, sl]), start=True, stop=True)
                    e = et_pool.tile([128, S], BF16, tag="et", name="et")
                    nc.scalar.activation(e[:], sc[:],
                                         mybir.ActivationFunctionType.Exp)
                    for nh in range(S // NHALF):
                        sl = slice(NHALF * nh, NHALF * (nh + 1))
                        nc.tensor.matmul(
                            av[:, sl],
                            _r(v_sb[kb][:, (HD + 1) * i:(HD + 1) * (i + 1)]),
                            _r(e[:, sl]),
                            start=(kb == 0), stop=(kb == NKB - 1))
                rowsum = small.tile([1, S], F32, tag="rowsum", name="rowsum",
                                    bufs=1)
                nc.scalar.copy(rowsum[:], av[HD:HD + 1, :])
                nc.sync.dma_start(bass.AP(rs_dram, 0, [[1, S]]), rowsum[:])
                rs_t = small.tile([128, NQB], F32, tag="rs_t", name="rs_t")
                nc.sync.dma_start(
                    rs_t[:], bass.AP(rs_dram, 0, [[1, 128], [128, NQB]]))
                rc_t = small.tile([128, NQB], F32, tag="rc_t", name="rc_t")
                nc.vector.reciprocal(rc_t[:], rs_t[:])
                nc.sync.dma_start(
                    bass.AP(rc_dram, 0, [[1, 128], [128, NQB]]), rc_t[:])
                recip = small.tile([1, S], F32, tag="recip", name="recip",
                                   bufs=1)
                nc.sync.dma_start(recip[:], bass.AP(rc_dram, 0, [[1, S]]))
                rb = small.tile([64, S], F32, tag="rbcast", name="rbcast",
                                bufs=1)
                nc.gpsimd.partition_broadcast(rb[:], recip[:])
                chunk, row = i // 2, (i % 2) * 64
                nc.vector.tensor_tensor(
                    out_headsT[chunk][row:row + 64, :], av[0:HD, :], rb[:],
                    op=mybir.AluOpType.mult)

            # ---- output projection (partial) ----
            for qb in range(NQB):
                pp = psum_big.tile([128, C], F32, tag="big", name="pp")
                for ci in range(HPC * HD // 128):
                    nc.tensor.matmul(
                        pp[:, 0:NHALF],
                        _r(out_headsT[ci][:, 128 * qb:128 * (qb + 1)]),
                        _r(wproj_sb[ci][:, 0:NHALF]),
                        start=(ci == 0), stop=(ci == 2))
                    nc.tensor.matmul(
                        pp[:, NHALF:C],
                        _r(out_headsT[ci][:, 128 * qb:128 * (qb + 1)]),
                        _r(wproj_sb[ci][:, NHALF:C]),
                        start=(ci == 0), stop=(ci == 2))
                pp_sb = small.tile([128, C], F32, tag="pp_sb", name="pp_sb", bufs=1)
                (nc.scalar.copy if qb % 2 else nc.vector.tensor_copy)(
                    pp_sb[:], pp[:])
                nc.sync.dma_start(out[128 * qb:128 * (qb + 1), :], pp_sb[:])

    nc.compile()
    return nc


def shard_inputs(x, Wqkv, Wproj, rel_pos_h, rel_pos_w):
    """Build the 8 per-core input maps."""
    import ml_dtypes
    bf16 = ml_dtypes.bfloat16
    scale = HD ** (-0.5)
    x = np.asarray(x, dtype=np.float32)
    Wqkv = np.asarray(Wqkv, dtype=np.float32)
    Wproj = np.asarray(Wproj, dtype=np.float32)
    rhT = np.ascontiguousarray(np.asarray(rel_pos_h, np.float32).T).astype(bf16)
    rwT = np.ascontiguousarray(np.asarray(rel_pos_w, np.float32).T).astype(bf16)
    idb = np.zeros((2 * H - 1, 3 * W - 1), np.float32)
    for r in range(2 * H - 1):
        idb[r, r] = 1.0
    idb = idb.astype(bf16)
    oh = np.zeros((65, S), np.float32)
    for khp in range(H):
        oh[khp, (31 - khp) * W:(31 - khp) * W + W] = 1.0
    for kwp in range(W):
        oh[32 + kwp, 31 - kwp::W] = 1.0
    oh[64, :] = 1.0
    oh = oh.astype(bf16)
    in_maps = []
    for core in range(NCORES):
        b = core // 2
        h0 = (core % 2) * HPC
        xb = x[b].reshape(S, C)
        xT = np.ascontiguousarray(xb.T).astype(bf16)
        wq = Wqkv[:, h0 * HD:(h0 + HPC) * HD]
        wk = Wqkv[:, C + h0 * HD:C + (h0 + HPC) * HD] * scale
        wqk = np.ascontiguousarray(np.concatenate([wq, wk], axis=1)).astype(bf16)
        wv = np.ascontiguousarray(
            Wqkv[:, 2 * C + h0 * HD:2 * C + (h0 + HPC) * HD]).astype(bf16)
        wp = np.ascontiguousarray(Wproj[h0 * HD:(h0 + HPC) * HD, :]).astype(bf16)
        in_maps.append({"xT": xT, "wqk": wqk, "wv": wv, "wproj": wp,
                        "rhT": rhT, "rwT": rwT, "onehot": oh,
                        "idband": idb})
    return in_maps


_NC_CACHE = {}


def kernel(x, Wqkv, Wproj, bproj, rel_pos_h, rel_pos_w):
    if "nc" not in _NC_CACHE:
        _NC_CACHE["nc"] = build_program()
    nc = _NC_CACHE["nc"]
    in_maps = shard_inputs(x, Wqkv, Wproj, rel_pos_h, rel_pos_w)
    res = run_bass_kernel_spmd(nc, in_maps, list(range(NCORES)))
    bproj = np.asarray(bproj, dtype=np.float32)
    out = np.empty((B, H, W, C), dtype=np.float32)
    for b in range(B):
        acc = res.results[2 * b]["out"] + res.results[2 * b + 1]["out"] + bproj
        out[b] = acc.reshape(H, W, C)
    return out



# revision 24
# speedup vs baseline: 1.2146x; 1.2146x over previous
"""Trainium2 Bass kernel for ViTDet-style attention with decomposed
relative-position bias.

Problem shapes (hardcoded):
  x: (4, 32, 32, 768) f32, Wqkv: (768, 2304), Wproj: (768, 768),
  bproj: (768,), rel_pos_h/w: (63, 64).
  12 heads, head_dim 64, S = 32*32 = 1024.

Sharding: 48 (batch, head) pairs -> 6 heads per core, all of one batch per
core-pair. Each core computes its heads' attention and a partial output
projection (its heads' channel rows of Wproj); the host sums the two
partials per batch and adds bproj.

Device algorithm per core (bf16 matmuls, fp32 PSUM accumulation):
  - v   = x @ Wv (natural layout, with an appended ones column per head)
  - qkT = Wqk^T @ x^T  (x^T supplied pre-transposed by host; k pre-scaled),
    written into two mega-tiles kaug_all/qaug_all [128, 6*1024].
  - rel-pos bias rows computed DIRECTLY from qT: for shift s,
    BhT[r, q in h-block s] = sum_c rhT[c, s+r] * qT[c, q]  (stationary is a
    32-col slice of the rel table; all 6 heads packed in one N=192 matmul
    via a 3D moving AP; 4 shifts accumulate into one PSUM tile so the
    PSUM->SBUF copy is one big strided op).
  - scoresT (k x q) = kaug^T @ qaug in ONE K=128 matmul per tile:
    aug rows 0-63 = kT / qT, 64-95 = one-hot(h) / BhT, 96-127 = one-hot(w)/BwT
    => rel-pos bias folded into the QK matmul for free.
  - eT = exp(scoresT): head A direct on ScalarE from PSUM; head B via DVE
    bf16 cast then 2x-rate ScalarE exp (keeps attention PE-bound).
  - avT (65 x q) accumulates v_aug^T-matmul over k blocks; row 64 = softmax
    denominator via the ones column.
  - normalize: DVE reciprocal of row 64, gpsimd partition-broadcast (all
    on-chip, no DRAM bounce), DVE multiply into out_headsT.
  - partial = out_heads @ Wproj_shard  (natural layout, DMA SBUF->DRAM).
"""

import numpy as np

import concourse.bass as bass
import concourse.bacc as bacc
import concourse.mybir as mybir
import concourse.tile as tile
from concourse.tile import add_dep_helper
from concourse.bass_utils import run_bass_kernel_spmd

F32 = mybir.dt.float32
BF16 = mybir.dt.bfloat16
EXP = mybir.ActivationFunctionType.Exp

NH = 12          # total heads
C = 768
HD = 64
H = W = 32
S = H * W        # 1024
B = 4
NCORES = 8
HPC = NH * B // NCORES   # heads per core = 6
NCH = 6                  # C // 128 input-channel chunks
NKB = S // 128           # 8 k blocks
NQB = S // 128           # 8 q blocks
NHALF = 512              # matmul moving-dim half
AW = HPC * S             # mega-tile width 6144


def build_program(dbg=False):
    nc = bacc.Bacc("TRN2", target_bir_lowering=False, debug=False)

    xT = nc.declare_dram_parameter("xT", [C, S], BF16, isOutput=False)
    wqk = nc.declare_dram_parameter("wqk", [C, 2 * HPC * HD], BF16, isOutput=False)
    wv = nc.declare_dram_parameter("wv", [C, HPC * HD], BF16, isOutput=False)
    wproj = nc.declare_dram_parameter("wproj", [HPC * HD, C], BF16, isOutput=False)
    rhT = nc.declare_dram_parameter("rhT", [HD, 2 * H - 1], BF16, isOutput=False)
    rwT = nc.declare_dram_parameter("rwT", [HD, 2 * W - 1], BF16, isOutput=False)
    onehot = nc.declare_dram_parameter("onehot", [65, S], BF16, isOutput=False)
    out = nc.declare_dram_parameter("out", [S, C], F32, isOutput=True)

    with tile.TileContext(nc) as tc:
        with (
            tc.tile_pool(name="persist", bufs=1) as persist,
            tc.tile_pool(name="psc", bufs=2, space="PSUM") as psc,
            tc.tile_pool(name="pav", bufs=2, space="PSUM") as pav,
            tc.tile_pool(name="et", bufs=4) as et_pool,
            tc.tile_pool(name="small", bufs=2) as small,
        ):
            # ---- persistent SBUF loads, in consumption order ----
            xT_sb = []
            wv_sb = []
            for ci in range(NCH):
                t = persist.tile([128, S], BF16, tag=f"xT{ci}", name=f"xT{ci}")
                nc.sync.dma_start(t[:], xT[128 * ci:128 * (ci + 1), :])
                xT_sb.append(t)
                t = persist.tile([128, HPC * HD], BF16, tag=f"wv{ci}", name=f"wv{ci}")
                nc.sync.dma_start(t[:], wv[128 * ci:128 * (ci + 1), :])
                wv_sb.append(t)
            wqk_sb = []
            for ci in range(NCH):
                t = persist.tile([128, 2 * HPC * HD], BF16, tag=f"wqk{ci}", name=f"wqk{ci}")
                nc.sync.dma_start(t[:], wqk[128 * ci:128 * (ci + 1), :])
                wqk_sb.append(t)
            oh = persist.tile([65, S], BF16, tag="onehot", name="onehot")
            nc.sync.dma_start(oh[:], onehot[:, :])
            rhT_sb = persist.tile([HD, 2 * H - 1], BF16, tag="rhT", name="rhT_sb")
            nc.sync.dma_start(rhT_sb[:], rhT[:, :])
            rwT_sb = persist.tile([HD, 2 * W - 1], BF16, tag="rwT", name="rwT_sb")
            nc.sync.dma_start(rwT_sb[:], rwT[:, :])
            wproj_sb = []
            for ci in range(HPC * HD // 128):
                t = persist.tile([128, C], BF16, tag=f"wproj{ci}", name=f"wproj{ci}")
                nc.sync.dma_start(t[:], wproj[128 * ci:128 * (ci + 1), :])
                wproj_sb.append(t)

            # ---- augmented k/q mega-tiles [128, 6*1024] ----
            kaug = persist.tile([128, AW], BF16, tag="kaug", name="kaug")
            qaug = persist.tile([128, AW], BF16, tag="qaug", name="qaug")
            # one-hot rows (constant) DMA'd straight into kaug rows 64-127
            for i in range(HPC):
                nc.sync.dma_start(kaug[64:128, S * i:S * (i + 1)], onehot[0:64, :])

            # ---- v projection (natural) + ones column ----
            # v_sb[sb]: (128, 6*65) cols [65i..65i+64) = head i v, col 65i+64 = 1
            # Manual (multi-dim strided) APs get imprecise subtile dep ranges,
            # so ordering edges for their readers/writers are added explicitly
            # below via add_dep_helper (engine program order covers the rest).
            vcopy_insts = []
            v_sb = [persist.tile([128, HPC * (HD + 1)], BF16, tag=f"v{sb}", name=f"v{sb}")
                    for sb in range(NKB)]
            for sb in range(NKB):
                vp = psc.tile([128, HPC * HD + HPC], F32, tag="big", name="vp")
                for ci in range(NCH):
                    nc.tensor.matmul(
                        vp[:, 0:HPC * HD],
                        xT_sb[ci][:, 128 * sb:128 * (sb + 1)],
                        wv_sb[ci][:],
                        start=(ci == 0), stop=(ci == NCH - 1))
                nc.tensor.matmul(vp[:, HPC * HD:HPC * HD + HPC],
                                 oh[64:65, 128 * sb:128 * (sb + 1)],
                                 oh[64:65, 0:HPC], start=True, stop=True)
                src = bass.AP(vp.tensor, vp[:].offset,
                              [vp[:].ap[0], [HD, HPC], [1, HD]])
                dst = bass.AP(v_sb[sb].tensor, v_sb[sb][:].offset,
                              [v_sb[sb][:].ap[0], [HD + 1, HPC], [1, HD]])
                ones_src = bass.AP(vp.tensor, vp[:].offset + HPC * HD,
                                   [vp[:].ap[0], [1, HPC]])
                ones_dst = bass.AP(v_sb[sb].tensor, v_sb[sb][:].offset + HD,
                                   [v_sb[sb][:].ap[0], [HD + 1, HPC]])
                if sb % 2:
                    vcopy_insts.append(nc.scalar.copy(dst, src))
                    vcopy_insts.append(nc.scalar.copy(ones_dst, ones_src))
                else:
                    vcopy_insts.append(nc.vector.tensor_copy(dst, src))
                    vcopy_insts.append(nc.vector.tensor_copy(ones_dst, ones_src))

            # ---- qk projection into the mega-tiles ----
            # octile t covers oc rows [128t, 128t+128): t<3 -> q, t>=3 -> k
            qcopy_insts = []
            for t in range(2 * HPC * HD // 128):
                qp = psc.tile([128, S], F32, tag="big", name="qp")
                for ci in range(NCH):
                    for nh in range(S // NHALF):
                        nc.tensor.matmul(
                            qp[:, NHALF * nh:NHALF * (nh + 1)],
                            wqk_sb[ci][:, 128 * t:128 * (t + 1)],
                            xT_sb[ci][:, NHALF * nh:NHALF * (nh + 1)],
                            start=(ci == 0), stop=(ci == NCH - 1))
                for sub in range(2):
                    head = (t % 3) * 2 + sub
                    dstt = qaug if t < 3 else kaug
                    dst_ap = dstt[0:64, S * head:S * (head + 1)]
                    src_ap = qp[64 * sub:64 * sub + 64, :]
                    if sub == 0:
                        cp = nc.scalar.copy(dst_ap, src_ap)
                    else:
                        cp = nc.vector.tensor_copy(dst_ap, src_ap)
                    if t < 3:
                        qcopy_insts.append(cp)

            # ---- rel-pos bias rows, direct from qT ----
            # For shift s (= h or w coordinate value), rows r in [0,32):
            #   qaug[64+r, q in block s of head i] = sum_c rhT[c, s+r]*qT[c, q]
            # 4 shifts accumulate into one [32, 768] PSUM tile -> one copy.
            qa64 = qaug[0:64, 0:1]
            band_copy_insts = []
            first_band_mm = None
            for axis in range(2):      # 0 = h, 1 = w
                tbl = rhT_sb if axis == 0 else rwT_sb
                for sq in range(8):    # shift quad: shifts 4sq .. 4sq+3
                    # each u block padded to 256 f32 so no matmul output
                    # crosses a 2KB PSUM bank boundary
                    bp = psc.tile([32, 4 * 256], F32, tag="big", name="bp")
                    for u in range(4):
                        s = 4 * sq + u
                        if axis == 0:
                            # h-block of head i: cols i*S + 32*s + j
                            rhs = bass.AP(qaug.tensor, qa64.offset + 32 * s,
                                          [qa64.ap[0], [S, HPC], [1, 32]])
                        else:
                            # w-block: cols i*S + s + 32*jh
                            rhs = bass.AP(qaug.tensor, qa64.offset + s,
                                          [qa64.ap[0], [S, HPC], [32, 32]])
                        mm = nc.tensor.matmul(bp[:, 256 * u:256 * u + 192],
                                              tbl[:, s:s + 32], rhs,
                                              start=True, stop=True)
                        if first_band_mm is None:
                            first_band_mm = mm
                            for cp in qcopy_insts:
                                add_dep_helper(mm.ins, cp.ins, sync=True,
                                               reason="band mm reads qT")
                    # one strided copy: psum cols u*192 + i*32 + j
                    if axis == 0:
                        dst = bass.AP(qaug.tensor,
                                      qaug[64:96, 0:1].offset + 128 * sq,
                                      [qaug[64:96, 0:1].ap[0],
                                       [S, HPC], [32, 4], [1, 32]])
                        src = bass.AP(bp.tensor, bp[:].offset,
                                      [bp[:].ap[0], [32, HPC], [256, 4], [1, 32]])
                        band_copy_insts.append(nc.scalar.copy(dst, src))
                    else:
                        dst = bass.AP(qaug.tensor,
                                      qaug[96:128, 0:1].offset + 4 * sq,
                                      [qaug[96:128, 0:1].ap[0],
                                       [S, HPC], [32, 32], [1, 4]])
                        src = bass.AP(bp.tensor, bp[:].offset,
                                      [bp[:].ap[0], [32, HPC], [1, 32], [256, 4]])
                        band_copy_insts.append(nc.vector.tensor_copy(dst, src))

            if dbg:
                qaug_dbg = nc.declare_dram_parameter("qaug_dbg", [128, AW], BF16,
                                                     isOutput=True)
                kaug_dbg = nc.declare_dram_parameter("kaug_dbg", [128, AW], BF16,
                                                     isOutput=True)
                vsb_dbg = nc.declare_dram_parameter("vsb_dbg",
                                                    [128, NKB * HPC * (HD + 1)],
                                                    BF16, isOutput=True)
                d1 = nc.sync.dma_start(qaug_dbg[:, :], qaug[:, :])
                for cp in band_copy_insts:
                    add_dep_helper(d1.ins, cp.ins, sync=True, reason="dbg")
                nc.sync.dma_start(kaug_dbg[:, :], kaug[:, :])
                for sb in range(NKB):
                    d2 = nc.sync.dma_start(
                        vsb_dbg[:, sb * 390:(sb + 1) * 390], v_sb[sb][:])
                    for cp in vcopy_insts:
                        add_dep_helper(d2.ins, cp.ins, sync=True, reason="dbg")

            # ---- attention, two heads in flight per pair ----
            out_hT = [persist.tile([128, S], BF16, tag=f"ohT{c}", name=f"ohT{c}")
                      for c in range(HPC * HD // 128)]

            state = {"first_sc": True, "first_av": True}

            def sc_mms(head, kb):
                scp = psc.tile([128, S], F32, tag="big", name="scp")
                for nh in range(S // NHALF):
                    mm = nc.tensor.matmul(
                        scp[:, NHALF * nh:NHALF * (nh + 1)],
                        kaug[:, S * head + 128 * kb:S * head + 128 * (kb + 1)],
                        qaug[:, S * head + NHALF * nh:S * head + NHALF * (nh + 1)],
                        start=True, stop=True)
                    if state["first_sc"]:
                        state["first_sc"] = False
                        for cp in band_copy_insts:
                            add_dep_helper(mm.ins, cp.ins, sync=True,
                                           reason="scores read band rows")
                return scp

            def av_mms(av, head, kb, e):
                for nh in range(S // NHALF):
                    mm = nc.tensor.matmul(
                        av[:, NHALF * nh:NHALF * (nh + 1)],
                        v_sb[kb][:, (HD + 1) * head:(HD + 1) * (head + 1)],
                        e[:, NHALF * nh:NHALF * (nh + 1)],
                        start=(kb == 0), stop=(kb == NKB - 1))
                    if state["first_av"]:
                        state["first_av"] = False
                        for cp in vcopy_insts:
                            add_dep_helper(mm.ins, cp.ins, sync=True,
                                           reason="av reads v_sb")

            def norm(av, head):
                recip = small.tile([1, S], F32, tag="recip", name="recip", bufs=2)
                nc.vector.reciprocal(recip[:], av[HD:HD + 1, :])
                rb = small.tile([64, S], F32, tag="rbcast", name="rbcast", bufs=2)
                nc.gpsimd.partition_broadcast(rb[:], recip[:])
                chunk, row = head // 2, (head % 2) * 64
                nc.vector.tensor_tensor(
                    out_hT[chunk][row:row + 64, :], av[0:HD, :], rb[:],
                    op=mybir.AluOpType.mult)

            for p in range(HPC // 2):
                ha, hb = 2 * p, 2 * p + 1
                av_a = pav.tile([HD + 1, S], F32, tag="av", name="av_a")
                av_b = pav.tile([HD + 1, S], F32, tag="av", name="av_b")
                ea = eb = ebprev = None
                for kb in range(NKB + 1):
                    if kb < NKB:
                        sca = sc_mms(ha, kb)
                        scb = sc_mms(hb, kb)
                    if kb > 0:
                        av_mms(av_a, ha, kb - 1, ea)
                    if kb > 1:
                        av_mms(av_b, hb, kb - 2, ebprev)
                    if kb < NKB:
                        # head A: direct exp on ScalarE from PSUM
                        ea = et_pool.tile([128, S], BF16, tag="et", name="ea")
                        nc.scalar.activation(ea[:], sca[:], EXP)
                        # head B: DVE bf16 cast, then 2x-rate ScalarE exp
                        sb16 = et_pool.tile([128, S], BF16, tag="sb16",
                                            name="sb16", bufs=2)
                        nc.vector.tensor_copy(sb16[:], scb[:])
                        ebprev = eb
                        eb = et_pool.tile([128, S], BF16, tag="et", name="eb")
                        nc.scalar.activation(eb[:], sb16[:], EXP)
                av_mms(av_b, hb, NKB - 1, eb)
                norm(av_a, ha)
                norm(av_b, hb)

            if dbg:
                oht_dbg = nc.declare_dram_parameter("oht_dbg", [128, 3 * S], BF16,
                                                    isOutput=True)
                for c in range(3):
                    nc.sync.dma_start(oht_dbg[:, S * c:S * (c + 1)], out_hT[c][:])

            # ---- output projection (partial) ----
            for qb in range(NQB):
                pp = psc.tile([128, C], F32, tag="big", name="pp")
                for ci in range(HPC * HD // 128):
                    nc.tensor.matmul(
                        pp[:, 0:NHALF],
                        out_hT[ci][:, 128 * qb:128 * (qb + 1)],
                        wproj_sb[ci][:, 0:NHALF],
                        start=(ci == 0), stop=(ci == 2))
                    nc.tensor.matmul(
                        pp[:, NHALF:C],
                        out_hT[ci][:, 128 * qb:128 * (qb + 1)],
                        wproj_sb[ci][:, NHALF:C],
                        start=(ci == 0), stop=(ci == 2))
                pp_sb = small.tile([128, C], F32, tag="pp_sb", name="pp_sb", bufs=2)
                (nc.scalar.copy if qb % 2 else nc.vector.tensor_copy)(
                    pp_sb[:], pp[:])
                nc.sync.dma_start(out[128 * qb:128 * (qb + 1), :], pp_sb[:])

    nc.compile()
    return nc


def shard_inputs(x, Wqkv, Wproj, rel_pos_h, rel_pos_w):
    """Build the 8 per-core input maps."""
    import ml_dtypes
    bf16 = ml_dtypes.bfloat16
    scale = HD ** (-0.5)
    x = np.asarray(x, dtype=np.float32)
    Wqkv = np.asarray(Wqkv, dtype=np.float32)
    Wproj = np.asarray(Wproj, dtype=np.float32)
    rhT = np.ascontiguousarray(np.asarray(rel_pos_h, np.float32).T).astype(bf16)
    rwT = np.ascontiguousarray(np.asarray(rel_pos_w, np.float32).T).astype(bf16)
    oh = np.zeros((65, S), np.float32)
    for khp in range(H):
        oh[khp, (31 - khp) * W:(31 - khp) * W + W] = 1.0
    for kwp in range(W):
        oh[32 + kwp, 31 - kwp::W] = 1.0
    oh[64, :] = 1.0
    oh = oh.astype(bf16)
    in_maps = []
    for core in range(NCORES):
        b = core // 2
        h0 = (core % 2) * HPC
        xb = x[b].reshape(S, C)
        xT = np.ascontiguousarray(xb.T).astype(bf16)
        wq = Wqkv[:, h0 * HD:(h0 + HPC) * HD]
        wk = Wqkv[:, C + h0 * HD:C + (h0 + HPC) * HD] * scale
        wqk = np.ascontiguousarray(np.concatenate([wq, wk], axis=1)).astype(bf16)
        wv = np.ascontiguousarray(
            Wqkv[:, 2 * C + h0 * HD:2 * C + (h0 + HPC) * HD]).astype(bf16)
        wp = np.ascontiguousarray(Wproj[h0 * HD:(h0 + HPC) * HD, :]).astype(bf16)
        in_maps.append({"xT": xT, "wqk": wqk, "wv": wv, "wproj": wp,
                        "rhT": rhT, "rwT": rwT, "onehot": oh})
    return in_maps


_NC_CACHE = {}


def kernel(x, Wqkv, Wproj, bproj, rel_pos_h, rel_pos_w):
    if "nc" not in _NC_CACHE:
        _NC_CACHE["nc"] = build_program()
    nc = _NC_CACHE["nc"]
    in_maps = shard_inputs(x, Wqkv, Wproj, rel_pos_h, rel_pos_w)
    res = run_bass_kernel_spmd(nc, in_maps, list(range(NCORES)))
    bproj = np.asarray(bproj, dtype=np.float32)
    out = np.empty((B, H, W, C), dtype=np.float32)
    for b in range(B):
        acc = res.results[2 * b]["out"] + res.results[2 * b + 1]["out"] + bproj
        out[b] = acc.reshape(H, W, C)
    return out


# revision 27
# speedup vs baseline: 1.5094x; 1.2428x over previous
"""Trainium2 Bass kernel for ViTDet-style attention with decomposed
relative-position bias.

Problem shapes (hardcoded):
  x: (4, 32, 32, 768) f32, Wqkv: (768, 2304), Wproj: (768, 768),
  bproj: (768,), rel_pos_h/w: (63, 64).
  12 heads, head_dim 64, S = 32*32 = 1024.

Sharding: 48 (batch, head) pairs -> 6 heads per core, all of one batch per
core-pair. Each core computes its heads' attention and a partial output
projection (its heads' channel rows of Wproj); the host sums the two
partials per batch and adds bproj.

Device algorithm per core (bf16 matmuls, fp32 PSUM accumulation):
  - v   = x @ Wv (natural layout, with an appended ones column per head)
  - qkT = Wqk^T @ x^T  (x^T supplied pre-transposed by host; k pre-scaled),
    written into two mega-tiles kaug_all/qaug_all [128, 6*1024].
  - rel-pos bias rows computed DIRECTLY from qT: for shift s,
    BhT[r, q in h-block s] = sum_c rhT[c, s+r] * qT[c, q]  (stationary is a
    32-col slice of the rel table; all 6 heads packed in one N=192 matmul
    via a 3D moving AP; 4 shifts accumulate into one PSUM tile so the
    PSUM->SBUF copy is one big strided op).
  - scoresT (k x q) = kaug^T @ qaug in ONE K=128 matmul per tile:
    aug rows 0-63 = kT / qT, 64-95 = one-hot(h) / BhT, 96-127 = one-hot(w)/BwT
    => rel-pos bias folded into the QK matmul for free.
  - eT = exp(scoresT): head A direct on ScalarE from PSUM; head B via DVE
    bf16 cast then 2x-rate ScalarE exp (keeps attention PE-bound).
  - avT (65 x q) accumulates v_aug^T-matmul over k blocks; row 64 = softmax
    denominator via the ones column.
  - normalize: DVE reciprocal of row 64, gpsimd partition-broadcast (all
    on-chip, no DRAM bounce), DVE multiply into out_headsT.
  - partial = out_heads @ Wproj_shard  (natural layout, DMA SBUF->DRAM).
"""

import numpy as np

import concourse.bass as bass
import concourse.bacc as bacc
import concourse.mybir as mybir
import concourse.tile as tile
from concourse.tile import add_dep_helper
from concourse.bass_utils import run_bass_kernel_spmd

F32 = mybir.dt.float32
BF16 = mybir.dt.bfloat16
EXP = mybir.ActivationFunctionType.Exp

NH = 12          # total heads
C = 768
HD = 64
H = W = 32
S = H * W        # 1024
B = 4
NCORES = 8
HPC = NH * B // NCORES   # heads per core = 6
NCH = 6                  # C // 128 input-channel chunks
NKB = S // 128           # 8 k blocks
NQB = S // 128           # 8 q blocks
NHALF = 512              # matmul moving-dim half
AW = HPC * S             # mega-tile width 6144


def build_program(dbg=False):
    nc = bacc.Bacc("TRN2", target_bir_lowering=False, debug=False)

    xT = nc.declare_dram_parameter("xT", [C, S], BF16, isOutput=False)
    wqk = nc.declare_dram_parameter("wqk", [C, 2 * HPC * HD], BF16, isOutput=False)
    wv = nc.declare_dram_parameter("wv", [C, HPC * HD], BF16, isOutput=False)
    wproj = nc.declare_dram_parameter("wproj", [HPC * HD, C], BF16, isOutput=False)
    rhT = nc.declare_dram_parameter("rhT", [HD, 2 * H - 1], BF16, isOutput=False)
    rwT = nc.declare_dram_parameter("rwT", [HD, 2 * W - 1], BF16, isOutput=False)
    onehot = nc.declare_dram_parameter("onehot", [65, S], BF16, isOutput=False)
    out = nc.declare_dram_parameter("out", [S, C], F32, isOutput=True)

    with tile.TileContext(nc) as tc:
        with (
            tc.tile_pool(name="persist", bufs=1) as persist,
            tc.tile_pool(name="psc", bufs=2, space="PSUM") as psc,
            tc.tile_pool(name="pav", bufs=2, space="PSUM") as pav,
            tc.tile_pool(name="et", bufs=4) as et_pool,
            tc.tile_pool(name="small", bufs=2) as small,
        ):
            # ---- persistent SBUF loads, in consumption order ----
            xT_sb = []
            wv_sb = []
            for ci in range(NCH):
                t = persist.tile([128, S], BF16, tag=f"xT{ci}", name=f"xT{ci}")
                nc.sync.dma_start(t[:], xT[128 * ci:128 * (ci + 1), :])
                xT_sb.append(t)
                t = persist.tile([128, HPC * HD], BF16, tag=f"wv{ci}", name=f"wv{ci}")
                nc.sync.dma_start(t[:], wv[128 * ci:128 * (ci + 1), :])
                wv_sb.append(t)
            wqk_sb = []
            for ci in range(NCH):
                t = persist.tile([128, 2 * HPC * HD], BF16, tag=f"wqk{ci}", name=f"wqk{ci}")
                nc.sync.dma_start(t[:], wqk[128 * ci:128 * (ci + 1), :])
                wqk_sb.append(t)
            oh = persist.tile([65, S], BF16, tag="onehot", name="onehot")
            nc.sync.dma_start(oh[:], onehot[:, :])
            rhT_sb = persist.tile([HD, 2 * H - 1], BF16, tag="rhT", name="rhT_sb")
            nc.sync.dma_start(rhT_sb[:], rhT[:, :])
            rwT_sb = persist.tile([HD, 2 * W - 1], BF16, tag="rwT", name="rwT_sb")
            nc.sync.dma_start(rwT_sb[:], rwT[:, :])
            wproj_sb = []
            for ci in range(HPC * HD // 128):
                t = persist.tile([128, C], BF16, tag=f"wproj{ci}", name=f"wproj{ci}")
                nc.sync.dma_start(t[:], wproj[128 * ci:128 * (ci + 1), :])
                wproj_sb.append(t)

            # ---- augmented k/q mega-tiles [128, 6*1024] ----
            kaug = persist.tile([128, AW], BF16, tag="kaug", name="kaug")
            qaug = persist.tile([128, AW], BF16, tag="qaug", name="qaug")
            # one-hot rows (constant) DMA'd straight into kaug rows 64-127
            for i in range(HPC):
                nc.sync.dma_start(kaug[64:128, S * i:S * (i + 1)], onehot[0:64, :])

            # ---- v projection (natural) + ones column ----
            # v_sb[sb]: (128, 6*65) cols [65i..65i+64) = head i v, col 65i+64 = 1
            # Manual (multi-dim strided) APs get imprecise subtile dep ranges,
            # so ordering edges for their readers/writers are added explicitly
            # below via add_dep_helper (engine program order covers the rest).
            vcopy_insts = []
            v_sb = [persist.tile([128, HPC * (HD + 1)], BF16, tag=f"v{sb}", name=f"v{sb}")
                    for sb in range(NKB)]
            for sb in range(NKB):
                vp = psc.tile([128, HPC * HD + HPC], F32, tag="big", name="vp")
                for ci in range(NCH):
                    nc.tensor.matmul(
                        vp[:, 0:HPC * HD],
                        xT_sb[ci][:, 128 * sb:128 * (sb + 1)],
                        wv_sb[ci][:],
                        start=(ci == 0), stop=(ci == NCH - 1))
                nc.tensor.matmul(vp[:, HPC * HD:HPC * HD + HPC],
                                 oh[64:65, 128 * sb:128 * (sb + 1)],
                                 oh[64:65, 0:HPC], start=True, stop=True)
                src = bass.AP(vp.tensor, vp[:].offset,
                              [vp[:].ap[0], [HD, HPC], [1, HD]])
                dst = bass.AP(v_sb[sb].tensor, v_sb[sb][:].offset,
                              [v_sb[sb][:].ap[0], [HD + 1, HPC], [1, HD]])
                ones_src = bass.AP(vp.tensor, vp[:].offset + HPC * HD,
                                   [vp[:].ap[0], [1, HPC]])
                ones_dst = bass.AP(v_sb[sb].tensor, v_sb[sb][:].offset + HD,
                                   [v_sb[sb][:].ap[0], [HD + 1, HPC]])
                if sb % 2:
                    vcopy_insts.append(nc.scalar.copy(dst, src))
                    vcopy_insts.append(nc.scalar.copy(ones_dst, ones_src))
                else:
                    vcopy_insts.append(nc.vector.tensor_copy(dst, src))
                    vcopy_insts.append(nc.vector.tensor_copy(ones_dst, ones_src))

            # ---- qk projection into the mega-tiles ----
            # octile t covers oc rows [128t, 128t+128): t<3 -> q, t>=3 -> k
            qcopy_insts = []
            for t in range(2 * HPC * HD // 128):
                qp = psc.tile([128, S], F32, tag="big", name="qp")
                for ci in range(NCH):
                    for nh in range(S // NHALF):
                        nc.tensor.matmul(
                            qp[:, NHALF * nh:NHALF * (nh + 1)],
                            wqk_sb[ci][:, 128 * t:128 * (t + 1)],
                            xT_sb[ci][:, NHALF * nh:NHALF * (nh + 1)],
                            start=(ci == 0), stop=(ci == NCH - 1))
                for sub in range(2):
                    head = (t % 3) * 2 + sub
                    dstt = qaug if t < 3 else kaug
                    dst_ap = dstt[0:64, S * head:S * (head + 1)]
                    src_ap = qp[64 * sub:64 * sub + 64, :]
                    if sub == 0:
                        cp = nc.scalar.copy(dst_ap, src_ap)
                    else:
                        cp = nc.vector.tensor_copy(dst_ap, src_ap)
                    if t < 3:
                        qcopy_insts.append(cp)

            # ---- rel-pos bias rows, direct from qT ----
            # For shift s (= h or w coordinate value), rows r in [0,32):
            #   qaug[64+r, q in block s of head i] = sum_c rhT[c, s+r]*qT[c, q]
            # 4 shifts accumulate into one [32, 768] PSUM tile -> one copy.
            qa64 = qaug[0:64, 0:1]
            band_copy_insts = []
            first_band_mm = None
            for axis in range(2):      # 0 = h, 1 = w
                tbl = rhT_sb if axis == 0 else rwT_sb
                for sq in range(8):    # shift quad: shifts 4sq .. 4sq+3
                    # each u block padded to 256 f32 so no matmul output
                    # crosses a 2KB PSUM bank boundary
                    bp = psc.tile([32, 4 * 256], F32, tag="big", name="bp")
                    for u in range(4):
                        s = 4 * sq + u
                        if axis == 0:
                            # h-block of head i: cols i*S + 32*s + j
                            rhs = bass.AP(qaug.tensor, qa64.offset + 32 * s,
                                          [qa64.ap[0], [S, HPC], [1, 32]])
                        else:
                            # w-block: cols i*S + s + 32*jh
                            rhs = bass.AP(qaug.tensor, qa64.offset + s,
                                          [qa64.ap[0], [S, HPC], [32, 32]])
                        mm = nc.tensor.matmul(bp[:, 256 * u:256 * u + 192],
                                              tbl[:, s:s + 32], rhs,
                                              start=True, stop=True)
                        if first_band_mm is None:
                            first_band_mm = mm
                            for cp in qcopy_insts:
                                add_dep_helper(mm.ins, cp.ins, sync=True,
                                               reason="band mm reads qT")
                    # one strided copy: psum cols u*192 + i*32 + j
                    if axis == 0:
                        dst = bass.AP(qaug.tensor,
                                      qaug[64:96, 0:1].offset + 128 * sq,
                                      [qaug[64:96, 0:1].ap[0],
                                       [S, HPC], [32, 4], [1, 32]])
                        src = bass.AP(bp.tensor, bp[:].offset,
                                      [bp[:].ap[0], [32, HPC], [256, 4], [1, 32]])
                        band_copy_insts.append(nc.scalar.copy(dst, src))
                    else:
                        dst = bass.AP(qaug.tensor,
                                      qaug[96:128, 0:1].offset + 4 * sq,
                                      [qaug[96:128, 0:1].ap[0],
                                       [S, HPC], [32, 32], [1, 4]])
                        src = bass.AP(bp.tensor, bp[:].offset,
                                      [bp[:].ap[0], [32, HPC], [1, 32], [256, 4]])
                        band_copy_insts.append(nc.vector.tensor_copy(dst, src))

            if dbg:
                qaug_dbg = nc.declare_dram_parameter("qaug_dbg", [128, AW], BF16,
                                                     isOutput=True)
                kaug_dbg = nc.declare_dram_parameter("kaug_dbg", [128, AW], BF16,
                                                     isOutput=True)
                vsb_dbg = nc.declare_dram_parameter("vsb_dbg",
                                                    [128, NKB * HPC * (HD + 1)],
                                                    BF16, isOutput=True)
                d1 = nc.sync.dma_start(qaug_dbg[:, :], qaug[:, :])
                for cp in band_copy_insts:
                    add_dep_helper(d1.ins, cp.ins, sync=True, reason="dbg")
                nc.sync.dma_start(kaug_dbg[:, :], kaug[:, :])
                for sb in range(NKB):
                    d2 = nc.sync.dma_start(
                        vsb_dbg[:, sb * 390:(sb + 1) * 390], v_sb[sb][:])
                    for cp in vcopy_insts:
                        add_dep_helper(d2.ins, cp.ins, sync=True, reason="dbg")

            # ---- attention, two heads in flight per pair ----
            out_hT = [persist.tile([128, S], BF16, tag=f"ohT{c}", name=f"ohT{c}")
                      for c in range(HPC * HD // 128)]

            state = {"first_sc": True, "first_av": True}

            def sc_mms(head, kb):
                scp = psc.tile([128, S], F32, tag="big", name="scp")
                for nh in range(S // NHALF):
                    mm = nc.tensor.matmul(
                        scp[:, NHALF * nh:NHALF * (nh + 1)],
                        kaug[:, S * head + 128 * kb:S * head + 128 * (kb + 1)],
                        qaug[:, S * head + NHALF * nh:S * head + NHALF * (nh + 1)],
                        start=True, stop=True)
                    if state["first_sc"]:
                        state["first_sc"] = False
                        for cp in band_copy_insts:
                            add_dep_helper(mm.ins, cp.ins, sync=True,
                                           reason="scores read band rows")
                return scp

            def av_mms(av, head, kb, e):
                for nh in range(S // NHALF):
                    mm = nc.tensor.matmul(
                        av[:, NHALF * nh:NHALF * (nh + 1)],
                        v_sb[kb][:, (HD + 1) * head:(HD + 1) * (head + 1)],
                        e[:, NHALF * nh:NHALF * (nh + 1)],
                        start=(kb == 0), stop=(kb == NKB - 1))
                    if state["first_av"]:
                        state["first_av"] = False
                        for cp in vcopy_insts:
                            add_dep_helper(mm.ins, cp.ins, sync=True,
                                           reason="av reads v_sb")

            def norm(av, head):
                drow = small.tile([1, S], F32, tag="drow", name="drow", bufs=2)
                nc.vector.tensor_copy(drow[:], av[HD:HD + 1, :])
                recip = small.tile([1, S], F32, tag="recip", name="recip", bufs=2)
                nc.vector.reciprocal_approx_fast(recip[:], drow[:])
                rb = small.tile([64, S], F32, tag="rbcast", name="rbcast", bufs=2)
                nc.gpsimd.partition_broadcast(rb[:], recip[:])
                chunk, row = head // 2, (head % 2) * 64
                nc.vector.tensor_tensor(
                    out_hT[chunk][row:row + 64, :], av[0:HD, :], rb[:],
                    op=mybir.AluOpType.mult)

            for p in range(HPC // 2):
                ha, hb = 2 * p, 2 * p + 1
                av_a = pav.tile([HD + 1, S], F32, tag="av", name="av_a")
                av_b = pav.tile([HD + 1, S], F32, tag="av", name="av_b")
                ea = eb = ebprev = None
                for kb in range(NKB + 1):
                    if kb < NKB:
                        sca = sc_mms(ha, kb)
                        scb = sc_mms(hb, kb)
                    if kb > 0:
                        av_mms(av_a, ha, kb - 1, ea)
                    if kb > 1:
                        av_mms(av_b, hb, kb - 2, ebprev)
                    if kb < NKB:
                        # direct exp on ScalarE from PSUM (attention is
                        # ScalarE-bound; activations exist only there)
                        ea = et_pool.tile([128, S], BF16, tag="et", name="ea")
                        nc.scalar.activation(ea[:], sca[:], EXP)
                        ebprev = eb
                        eb = et_pool.tile([128, S], BF16, tag="et", name="eb")
                        nc.scalar.activation(eb[:], scb[:], EXP)
                av_mms(av_b, hb, NKB - 1, eb)
                norm(av_a, ha)
                norm(av_b, hb)

            if dbg:
                oht_dbg = nc.declare_dram_parameter("oht_dbg", [128, 3 * S], BF16,
                                                    isOutput=True)
                for c in range(3):
                    nc.sync.dma_start(oht_dbg[:, S * c:S * (c + 1)], out_hT[c][:])

            # ---- output projection (partial) ----
            for qb in range(NQB):
                pp = psc.tile([128, C], F32, tag="big", name="pp")
                for ci in range(HPC * HD // 128):
                    nc.tensor.matmul(
                        pp[:, 0:NHALF],
                        out_hT[ci][:, 128 * qb:128 * (qb + 1)],
                        wproj_sb[ci][:, 0:NHALF],
                        start=(ci == 0), stop=(ci == 2))
                    nc.tensor.matmul(
                        pp[:, NHALF:C],
                        out_hT[ci][:, 128 * qb:128 * (qb + 1)],
                        wproj_sb[ci][:, NHALF:C],
                        start=(ci == 0), stop=(ci == 2))
                pp_sb = small.tile([128, C], F32, tag="pp_sb", name="pp_sb", bufs=2)
                (nc.scalar.copy if qb % 2 else nc.vector.tensor_copy)(
                    pp_sb[:], pp[:])
                nc.sync.dma_start(out[128 * qb:128 * (qb + 1), :], pp_sb[:])

    nc.compile()
    return nc


def shard_inputs(x, Wqkv, Wproj, rel_pos_h, rel_pos_w):
    """Build the 8 per-core input maps."""
    import ml_dtypes
    bf16 = ml_dtypes.bfloat16
    scale = HD ** (-0.5)
    x = np.asarray(x, dtype=np.float32)
    Wqkv = np.asarray(Wqkv, dtype=np.float32)
    Wproj = np.asarray(Wproj, dtype=np.float32)
    rhT = np.ascontiguousarray(np.asarray(rel_pos_h, np.float32).T).astype(bf16)
    rwT = np.ascontiguousarray(np.asarray(rel_pos_w, np.float32).T).astype(bf16)
    oh = np.zeros((65, S), np.float32)
    for khp in range(H):
        oh[khp, (31 - khp) * W:(31 - khp) * W + W] = 1.0
    for kwp in range(W):
        oh[32 + kwp, 31 - kwp::W] = 1.0
    oh[64, :] = 1.0
    oh = oh.astype(bf16)
    in_maps = []
    for core in range(NCORES):
        b = core // 2
        h0 = (core % 2) * HPC
        xb = x[b].reshape(S, C)
        xT = np.ascontiguousarray(xb.T).astype(bf16)
        wq = Wqkv[:, h0 * HD:(h0 + HPC) * HD]
        wk = Wqkv[:, C + h0 * HD:C + (h0 + HPC) * HD] * scale
        wqk = np.ascontiguousarray(np.concatenate([wq, wk], axis=1)).astype(bf16)
        wv = np.ascontiguousarray(
            Wqkv[:, 2 * C + h0 * HD:2 * C + (h0 + HPC) * HD]).astype(bf16)
        wp = np.ascontiguousarray(Wproj[h0 * HD:(h0 + HPC) * HD, :]).astype(bf16)
        in_maps.append({"xT": xT, "wqk": wqk, "wv": wv, "wproj": wp,
                        "rhT": rhT, "rwT": rwT, "onehot": oh})
    return in_maps


_NC_CACHE = {}


def kernel(x, Wqkv, Wproj, bproj, rel_pos_h, rel_pos_w):
    if "nc" not in _NC_CACHE:
        _NC_CACHE["nc"] = build_program()
    nc = _NC_CACHE["nc"]
    in_maps = shard_inputs(x, Wqkv, Wproj, rel_pos_h, rel_pos_w)
    res = run_bass_kernel_spmd(nc, in_maps, list(range(NCORES)))
    bproj = np.asarray(bproj, dtype=np.float32)
    out = np.empty((B, H, W, C), dtype=np.float32)
    for b in range(B):
        acc = res.results[2 * b]["out"] + res.results[2 * b + 1]["out"] + bproj
        out[b] = acc.reshape(H, W, C)
    return out
